# revision 18
# baseline (speedup 1.0000x reference)
"""Trainium2 Bass kernel for nn_DecoderBlock (dense transformer decoder block).

Sharding: data-parallel over batch (8 batch elements -> 8 NeuronCores), no
collectives. Each core computes one full decoder block on [S=1024, D=1024].

Per-core strategy (bf16 datapath), v2 "two-stream pipeline":
  - activations feature-major ([D, S]) bf16; Linear = W_tile.T @ actT
  - after SA K/V are built, the downstream chain (SA softmax -> O-proj ->
    LN2 -> CA -> LN3 -> FFN -> output) is split into two independent
    query-column streams (cols 0-511 / 512-1023).  The ACT-bound softmax of
    one stream overlaps the PE-bound projections/FFN of the other.
  - attention head PAIRS issue their K=64 score matmuls back-to-back; bass
    auto-derives tile_position (0,0)/(64,0) from the kT base partitions, so
    the two matmuls run concurrently in disjoint PE row groups.
  - per (head, stream) the attn@V accumulator packs the two 256-col query
    quarters into ONE PSUM bank; softmax denominators ride as V's appended
    ones-column (row 64).  One reciprocal + one broadcast matmul + one
    PSUM*PSUM multiply per (head, stream) normalizes rep.
  - causal masking via gpsimd affine_select on the idle Pool engine.
  - LN rstd computed as exp(-0.5*ln(var+eps)) so the only ACT tables used
    are natural_log_exp (softmax exp shares it) and gelu.
  - buffers are aggressively reused across phases (rep lives in the dead
    halves of qT/xT buffers; FFN hidden lives in xT + CA-qT).
"""
import sys

sys.path.insert(0, '/opt/trn_rl_repo')

import contextlib

import ml_dtypes
import numpy as np

import concourse.bacc as bacc
import concourse.mybir as mybir
import concourse.tile as tile
from concourse.bass_utils import run_bass_kernel_spmd
from concourse.masks import make_identity

f32 = mybir.dt.float32
f32r = mybir.dt.float32r
bf16 = mybir.dt.bfloat16
AF = mybir.ActivationFunctionType
ALU = mybir.AluOpType

B, S, D, H, HD, FF = 8, 1024, 1024, 16, 64, 4096
ST = S // 128   # 8
DT = D // 128   # 8
FT = FF // 128  # 32
EPS = 1e-5
ISQ = 1.0 / 8.0  # 1/sqrt(HD)

W_NAMES = ['sa_wq', 'sa_wk', 'sa_wv', 'sa_wo', 'ca_wq', 'ca_wk', 'ca_wv', 'ca_wo']
B_NAMES = ['sa_bq', 'sa_bk', 'sa_bv', 'sa_bo', 'ca_bq', 'ca_bk', 'ca_bv', 'ca_bo']
LN_NAMES = ['ln1_g', 'ln1_b', 'ln2_g', 'ln2_b', 'ln3_g', 'ln3_b']


def _build(iters=1):
    nc = bacc.Bacc("TRN2", target_bir_lowering=False, debug=False, num_devices=8)

    dec_d = nc.dram_tensor("decoder", [S, D], f32, kind="ExternalInput").ap()
    enc_d = nc.dram_tensor("encoder", [S, D], f32, kind="ExternalInput").ap()
    wd = {n: nc.dram_tensor(n, [D, D], bf16, kind="ExternalInput").ap() for n in W_NAMES}
    bd = {n: nc.dram_tensor(n, [1, D] if n.endswith('bv') else [D], f32,
                            kind="ExternalInput").ap() for n in B_NAMES}
    lnd = {n: nc.dram_tensor(n, [D], f32, kind="ExternalInput").ap() for n in LN_NAMES}
    w1_d = nc.dram_tensor("ffn_w1", [D, FF], bf16, kind="ExternalInput").ap()
    b1_d = nc.dram_tensor("ffn_b1", [FF], f32, kind="ExternalInput").ap()
    w2_d = nc.dram_tensor("ffn_w2", [FF, D], bf16, kind="ExternalInput").ap()
    b2_d = nc.dram_tensor("ffn_b2", [D], f32, kind="ExternalInput").ap()
    out_d = nc.dram_tensor("out", [S, D], f32, kind="ExternalOutput").ap()

    with tile.TileContext(nc) as tc, \
            nc.allow_low_precision(reason="bf16 matmul pipeline by design"):
        _body(nc, tc, dec_d, enc_d, wd, bd, lnd, w1_d, b1_d, w2_d, b2_d, out_d, iters)
    nc.compile()
    return nc


def _body(nc, tc, dec_d, enc_d, wd, bd, lnd, w1_d, b1_d, w2_d, b2_d, out_d, iters):
    ctx = contextlib.ExitStack()
    with ctx:
        persist = ctx.enter_context(tc.tile_pool(name="persist", bufs=1))
        big = ctx.enter_context(tc.tile_pool(name="big", bufs=1))
        wf = ctx.enter_context(tc.tile_pool(name="wf", bufs=2))
        stri = ctx.enter_context(tc.tile_pool(name="stri", bufs=2))
        att = ctx.enter_context(tc.tile_pool(name="att", bufs=3))
        sm = ctx.enter_context(tc.tile_pool(name="sm", bufs=2))
        ps_a = ctx.enter_context(tc.tile_pool(name="ps_a", bufs=3, space="PSUM"))
        ps_r = ctx.enter_context(tc.tile_pool(name="ps_r", bufs=3, space="PSUM"))
        ps_s = ctx.enter_context(tc.tile_pool(name="ps_s", bufs=2, space="PSUM"))

        # ---- persistent constants ----
        ones_f = persist.tile([128, 8], f32, tag="ones_f")
        nc.vector.memset(ones_f, 1.0)
        ones_col_b = persist.tile([128, 1], bf16, tag="ones_col_b")
        nc.vector.tensor_copy(ones_col_b, ones_f[:, 0:1])
        ones_row = persist.tile([1, 128], f32r, tag="ones_row")
        ones_row_b = persist.tile([1, 128], bf16, tag="ones_row_b")
        ident_r = persist.tile([128, 128], f32r, tag="ident_r")
        ident_b = persist.tile([128, 128], bf16, tag="ident_b")
        ident_s = sm.tile([128, 256], f32, tag="sqf", name="ident_s")
        nc.vector.memset(ident_s[0:1, 0:128], 1.0)
        nc.vector.tensor_copy(ones_row, ident_s[0:1, 0:128])
        nc.vector.tensor_copy(ones_row_b, ident_s[0:1, 0:128])
        make_identity(nc, ident_s[:, 0:128])
        nc.vector.tensor_copy(ident_r, ident_s[:, 0:128])
        nc.vector.tensor_copy(ident_b, ident_s[:, 0:128])
        eps1 = persist.tile([1, 1], f32, tag="eps1")
        nc.vector.memset(eps1, EPS)
        eps128 = persist.tile([128, 1], f32, tag="eps128")
        nc.vector.memset(eps128, EPS)

        # LN row scratch (shared; Tile serializes the tiny row ops if both
        # streams hit an LN at once)
        rowA = persist.tile([1, 512], f32, tag="rowA")
        rowB = persist.tile([1, 512], f32, tag="rowB")
        rowC = persist.tile([1, 512], f32, tag="rowC")
        rowR1 = persist.tile([1, 512], f32r, tag="rowR1")   # rstd (f32r-only)
        rowR2 = persist.tile([1, 512], f32r, tag="rowR2")   # -mu*rstd (f32r-only)

        # causal quarter-masks: cmask[d][p, f] = (f - 128*d - p >= 0), [128,256]
        cmask = []
        for i in range(2):
            mf = sm.tile([128, 256], f32, tag="sqf", name=f"cmaskf{i}")
            nc.vector.memset(mf, 1.0)
            nc.gpsimd.affine_select(
                out=mf, in_=mf, compare_op=ALU.is_ge, fill=0.0,
                base=-128 * i, pattern=[[1, 256]], channel_multiplier=-1)
            mb = persist.tile([128, 256], bf16, tag=f"cmask{i}", name=f"cmask{i}")
            nc.vector.tensor_copy(mb, mf)
            cmask.append(mb)

        bias_t = {}
        for n in ['sa_bq', 'sa_bk', 'sa_bo', 'ca_bq', 'ca_bk', 'ca_bo']:
            bias_t[n] = persist.tile([128, DT], f32, tag=n, name=n)
            nc.sync.dma_start(bias_t[n], bd[n].rearrange("(t p) -> p t", p=128))
        for n in LN_NAMES:
            bias_t[n] = persist.tile([128, DT], f32, tag=n, name=n)
            nc.sync.dma_start(bias_t[n], lnd[n].rearrange("(t p) -> p t", p=128))
        bias_t['ffn_b1'] = persist.tile([128, FT], f32, tag="ffn_b1", name="ffn_b1")
        nc.sync.dma_start(bias_t['ffn_b1'], b1_d.rearrange("(t p) -> p t", p=128))
        bias_t['ffn_b2'] = persist.tile([128, DT], f32, tag="ffn_b2", name="ffn_b2")
        nc.sync.dma_start(bias_t['ffn_b2'], b2_d.rearrange("(t p) -> p t", p=128))

        # V-bias broadcast to all partitions (init-time)
        bcv_sb = {}
        for pre in ('sa_', 'ca_'):
            btmp = stri.tile([1, S], f32, tag="stripe_f", name=f"{pre}bvtmp")
            nc.sync.dma_start(btmp, bd[pre + 'bv'])
            brow = persist.tile([1, D], bf16, tag=f"{pre}bvrow", name=f"{pre}bvrow")
            nc.vector.tensor_copy(brow, btmp)
            bct = persist.tile([128, D], bf16, tag=f"{pre}bcv", name=f"{pre}bcv")
            for c in range(2):
                cs = slice(c * 512, (c + 1) * 512)
                bc = ps_a.tile([128, 512], f32, tag="a")
                nc.tensor.matmul(bc, ones_row_b, brow[:, cs],
                                 start=True, stop=True, skip_group_check=True)
                nc.vector.tensor_copy(bct[:, cs], bc)
            bcv_sb[pre] = bct

        # ---- big bf16 activation buffers [128, 8, 1024] (2 MB each) ----
        # A:  xT (LN1 out) -> CA repT -> FFN h[0:8]     (per-stream cols)
        # Bb: CA kT (written from encT, read by both streams)
        # C:  SA qT -> CA dst y2T/zT                    (per-stream cols)
        # Hb: SA kT
        # E:  encT -> SA dst x2T/yT -> FFN accumulator (outT)
        # Q2: SA repT -> CA qT -> FFN h[8:16]           (per-stream cols)
        A = big.tile([128, DT, S], bf16, tag="A")
        Bb = big.tile([128, DT, S], bf16, tag="Bb")
        C = big.tile([128, DT, S], bf16, tag="C")
        Hb = big.tile([128, DT, S], bf16, tag="Hb")
        E = big.tile([128, DT, S], bf16, tag="E")
        Q2 = big.tile([128, DT, S], bf16, tag="Q2")
        vg = big.tile([128, ST, H, 65], bf16, tag="vg")
        vg2 = big.tile([128, ST, H, 65], bf16, tag="vg2")
        for vx in (vg, vg2):
            for _st in range(ST):
                for _hg in range(2):
                    nc.vector.tensor_copy(vx[:, _st, _hg * 8:(_hg + 1) * 8, 64:65],
                                          ones_f[:, 0:8].unsqueeze(2))

        def mm(out_ap, lhsT_ap, rhs_ap, start, stop):
            nc.tensor.matmul(out_ap, lhsT_ap, rhs_ap, start=start, stop=stop,
                             skip_group_check=True)

        def load_w_full(w_dram, col0, ncols, tag="wf"):
            t = wf.tile([128, DT, 1024], bf16, tag=tag)
            nc.sync.dma_start(
                t[:, :, 0:ncols],
                w_dram[:, col0:col0 + ncols].rearrange("(k p) q -> p k q", p=128))
            return t

        def proj_full(wt, bias, src_T, dst_T):
            # both column halves, lhsT shared across the two c-chunks per k
            for m in range(DT):
                pss = [ps_a.tile([128, 512], f32, tag="a", name=f"pj{_c}")
                       for _c in range(2)]
                for k in range(DT):
                    for c in range(2):
                        cs = slice(c * 512, (c + 1) * 512)
                        mm(pss[c], wt[:, k, m * 128:(m + 1) * 128],
                           src_T[:, k, cs], k == 0, k == DT - 1)
                for c in range(2):
                    cs = slice(c * 512, (c + 1) * 512)
                    nc.vector.tensor_scalar(
                        dst_T[:, m, cs], pss[c], bias[:, m:m + 1], None, ALU.add)

        def proj_m(wt, bias, src_T, dst_T, c, m, residual=None):
            cs = slice(c * 512, (c + 1) * 512)
            pss = ps_a.tile([128, 512], f32, tag="a", name="pjs")
            for k in range(DT):
                mm(pss, wt[:, k, m * 128:(m + 1) * 128],
                   src_T[:, k, cs], k == 0, k == DT - 1)
            if residual is None:
                nc.vector.tensor_scalar(
                    dst_T[:, m, cs], pss, bias[:, m:m + 1], None, ALU.add)
            else:
                nc.vector.scalar_tensor_tensor(
                    dst_T[:, m, cs], pss, bias[:, m:m + 1],
                    residual[:, m, cs], ALU.add, ALU.add)

        def proj_stream(wt, bias, src_T, dst_T, c, residual=None):
            for m in range(DT):
                proj_m(wt, bias, src_T, dst_T, c, m, residual=residual)

        def vproj_full(wvt, src_kv_T, bcv, vgx):
            # V in natural layout (+bias broadcast, +ones column already set)
            for st in range(ST):
                for c in range(2):
                    cs = slice(c * 512, (c + 1) * 512)
                    psv = ps_a.tile([128, 512], f32, tag="a", name="psv")
                    for k in range(DT):
                        mm(psv, src_kv_T[:, k, st * 128:(st + 1) * 128],
                           wvt[:, k, cs], k == 0, k == DT - 1)
                    nc.vector.tensor_add(
                        vgx[:, st, c * 8:(c + 1) * 8, 0:64],
                        psv.rearrange("p (h e) -> p h e", h=8),
                        bcv[:, cs].rearrange("p (h e) -> p h e", h=8))

        def ln_stream(T, g_ap, b_ap, c):
            # in-place layernorm over the feature (partition-tiled) dim,
            # column range = stream c
            cs = slice(c * 512, (c + 1) * 512)
            sums = ps_a.tile([1, 512], f32, tag="a", name="sums")
            ssqs = ps_a.tile([1, 512], f32, tag="a", name="ssqs")
            for t in range(DT):
                sq = sm.tile([128, 512], bf16, tag="sqb", name="sq")
                nc.vector.tensor_mul(sq, T[:, t, cs], T[:, t, cs])
                mm(sums, ones_col_b, T[:, t, cs], t == 0, t == DT - 1)
                mm(ssqs, ones_col_b, sq, t == 0, t == DT - 1)
            nc.vector.tensor_scalar(rowA, sums, 1.0 / D, None, ALU.mult)    # mu
            nc.vector.tensor_scalar(rowB, ssqs, 1.0 / D, None, ALU.mult)    # E[x^2]
            nc.vector.scalar_tensor_tensor(rowC, rowA, -1.0, rowA, ALU.mult,
                                           ALU.mult)                        # -mu^2
            nc.vector.tensor_add(rowB, rowB, rowC)                          # var
            nc.scalar.activation(rowC, rowB, AF.Ln, bias=eps1)              # ln(var+eps)
            nc.scalar.activation(rowB, rowC, AF.Exp, scale=0.5)             # std
            nc.vector.reciprocal(rowR1, rowB)                               # rstd (f32r)
            nc.vector.scalar_tensor_tensor(rowR2, rowA, -1.0, rowR1,
                                           ALU.mult, ALU.mult)              # -mu*rstd
            bcA = ps_a.tile([128, 512], f32, tag="a", name="bcA")
            bcC = ps_a.tile([128, 512], f32, tag="a", name="bcC")
            mm(bcA, ones_row, rowR1, True, True)
            mm(bcC, ones_row, rowR2, True, True)
            for t in range(DT):
                tmp = sm.tile([128, 512], bf16, tag="lntmp", name="lntmp")
                nc.scalar.activation(tmp, bcC, AF.Identity,
                                     bias=b_ap[:, t:t + 1],
                                     scale=g_ap[:, t:t + 1])
                nc.vector.tensor_mul(T[:, t, cs], T[:, t, cs], bcA)
                nc.vector.scalar_tensor_tensor(
                    T[:, t, cs], T[:, t, cs], g_ap[:, t:t + 1],
                    tmp, ALU.mult, ALU.add)

        def attn_stream(c, qT, kT, vgx, causal, repT, fill=None):
            # softmax + attn@V for query columns [c*512, (c+1)*512)
            cs = slice(c * 512, (c + 1) * 512)
            n_skt = 4 if (causal and c == 0) else ST
            for hp in range(H // 2):
                h0 = 2 * hp
                rps = [ps_r.tile([128, 512], f32, tag="r", name=f"rp{_i}")
                       for _i in range(2)]
                started = [False, False]
                for skt in range(n_skt):
                    ats = [None, None]
                    scs = []
                    for i, po in enumerate((0, 64)):
                        sc = ps_s.tile([128, 512], f32, tag="s", name=f"sc{i}")
                        mm(sc, kT[po:po + 64, hp, skt * 128:(skt + 1) * 128],
                           qT[po:po + 64, hp, cs], True, True)
                        scs.append(sc)
                    d = skt * 128 - c * 512
                    for i in range(2):
                        at = att.tile([128, 512], bf16, tag="at", name=f"at{i}")
                        nc.scalar.activation(at, scs[i], AF.Exp, scale=ISQ)
                        if causal and 0 <= d < 512:
                            # mask only the diagonal-crossing 256-col quarter
                            qq = d // 256
                            sl = slice(qq * 256, (qq + 1) * 256)
                            nc.vector.tensor_mul(at[:, sl], at[:, sl],
                                                 cmask[(d % 256) // 128])
                        ats[i] = at
                    for q in range(2):
                        qlo = c * 512 + q * 256
                        if causal and skt * 128 > qlo + 255:
                            continue
                        is_last = (skt == n_skt - 1) and (q == 1)
                        for i in range(2):
                            mm(rps[i][0:65, q * 256:(q + 1) * 256],
                               vgx[:, skt, h0 + i, 0:65],
                               ats[i][:, q * 256:(q + 1) * 256],
                               not started[i], is_last)
                            started[i] = True
                # normalization: one recip + one bcast mm + one PSUM*PSUM mul
                # per head (covers both query quarters)
                for i in range(2):
                    ha = h0 + i
                    rec = sm.tile([1, 512], bf16, tag="rec", name="rec")
                    nc.vector.reciprocal(rec, rps[i][64:65, :])
                    bcr = ps_s.tile([128, 512], f32, tag="s", name="bcr")
                    mm(bcr[0:64, :], ones_row_b[:, 0:64], rec, True, True)
                    bcr_sb = sm.tile([64, 512], bf16, tag="bcr_sb", name="bcr_sb")
                    nc.vector.tensor_copy(bcr_sb, bcr[0:64, :])
                    nc.vector.tensor_mul(
                        repT[(ha % 2) * 64:(ha % 2) * 64 + 64, ha // 2, cs],
                        rps[i][0:64, :], bcr_sb)
                if fill is not None:
                    fill(hp)

        def enc_stripe(st):
            stf = stri.tile([128, S], f32, tag="stripe_f", name="encstf")
            nc.sync.dma_start(stf.bitcast(f32r),
                              enc_d[st * 128:(st + 1) * 128, :].bitcast(f32r))
            for j in range(DT):
                tp = ps_a.tile([128, 512], f32, tag="a", name="enctp")
                nc.tensor.transpose(tp[:, 0:128].bitcast(f32r),
                                    stf[:, j * 128:(j + 1) * 128].bitcast(f32r),
                                    ident_r)
                nc.vector.tensor_copy(E[:, j, st * 128:(st + 1) * 128],
                                      tp[:, 0:128])

        def ffn_chunk(c, idx):
            # idx 0..7: per hf-half (idx//4): two w1 cc-chunks then two w2
            # mg-chunks
            cs = slice(c * 512, (c + 1) * 512)
            hf, sub = idx // 4, idx % 4
            if sub < 2:
                cc = sub
                w1t = load_w_full(w1_d, (hf * 2 + cc) * 1024, 1024)
                for mi8 in range(8):
                    ft = hf * 16 + cc * 8 + mi8
                    loc = cc * 8 + mi8
                    pss = ps_a.tile([128, 512], f32, tag="a", name="f1")
                    for k in range(DT):
                        mm(pss, w1t[:, k, mi8 * 128:(mi8 + 1) * 128],
                           C[:, k, cs], k == 0, k == DT - 1)
                    hb, hslot = (A, loc) if loc < 8 else (Q2, loc - 8)
                    nc.scalar.activation(hb[:, hslot, cs], pss, AF.Gelu,
                                         bias=bias_t['ffn_b1'][:, ft:ft + 1],
                                         scale=1.0)
            else:
                mg = sub - 2
                w2g = wf.tile([128, 16, 512], bf16, tag="wf", name="w2g")
                nc.sync.dma_start(
                    w2g, w2_d[hf * 2048:(hf + 1) * 2048,
                              mg * 512:(mg + 1) * 512]
                    .rearrange("(k p) q -> p k q", p=128))
                for m4 in range(4):
                    m = mg * 4 + m4
                    pss = ps_a.tile([128, 512], f32, tag="a", name="f2")
                    for k2 in range(16):
                        hb, hslot = (A, k2) if k2 < 8 else (Q2, k2 - 8)
                        mm(pss, w2g[:, k2, m4 * 128:(m4 + 1) * 128],
                           hb[:, hslot, cs], k2 == 0, k2 == 15)
                    if hf == 0:
                        nc.vector.scalar_tensor_tensor(
                            E[:, m, cs], pss, 1.0, C[:, m, cs],
                            ALU.mult, ALU.add)
                    else:
                        nc.vector.scalar_tensor_tensor(
                            E[:, m, cs], pss, bias_t['ffn_b2'][:, m:m + 1],
                            E[:, m, cs], ALU.add, ALU.add)

        def ffn_stream(c):
            for idx in range(8):
                ffn_chunk(c, idx)

        def out_stream(c):
            for j in range(c * 4, c * 4 + 4):
                ost = stri.tile([128, S], f32, tag="stripe_f", name="ost")
                for i in range(DT):
                    tp = ps_a.tile([128, 512], f32, tag="a", name="otp")
                    tpb = tp.bitcast(bf16)
                    nc.tensor.transpose(tpb[:, 0:128],
                                        E[:, i, j * 128:(j + 1) * 128], ident_b)
                    nc.vector.tensor_copy(ost[:, i * 128:(i + 1) * 128],
                                          tpb[:, 0:128])
                nc.sync.dma_start(out_d[j * 128:(j + 1) * 128, :], ost)

        # ================= block body =================
        def block_body(_i=None):
            # P1: decoder stripes + LN1, transpose -> xT (A)
            for st in range(ST):
                stf = stri.tile([128, S], f32, tag="stripe_f", name="decstf")
                nc.sync.dma_start(stf, dec_d[st * 128:(st + 1) * 128, :])
                stats = sm.tile([128, 2, 6], f32, tag="bnst")
                xr = stf.rearrange("p (g d) -> p g d", g=2)
                for g2 in range(2):
                    nc.vector.bn_stats(stats[:, g2, :], xr[:, g2, :])
                mv = sm.tile([128, 2], f32, tag="bnmv")
                nc.vector.bn_aggr(mv, stats)
                lnv = sm.tile([128, 1], f32, tag="bnlnv")
                nc.scalar.activation(lnv, mv[:, 1:2], AF.Ln, bias=eps128)
                rstd = sm.tile([128, 1], f32, tag="bnrstd")
                nc.scalar.activation(rstd, lnv, AF.Exp, scale=-0.5)
                stb = stri.tile([128, S], bf16, tag="stripe_bf", name="decstb")
                nc.vector.tensor_scalar(stb, stf, mv[:, 0:1], rstd,
                                        ALU.subtract, ALU.mult)
                for j in range(DT):
                    tp = ps_a.tile([128, 512], f32, tag="a", name="dectp")
                    tpb = tp.bitcast(bf16)
                    nc.tensor.transpose(tpb[:, 0:128],
                                        stb[:, j * 128:(j + 1) * 128], ident_b)
                    nc.vector.tensor_scalar(
                        A[:, j, st * 128:(st + 1) * 128], tpb[:, 0:128],
                        bias_t['ln1_g'][:, j:j + 1], bias_t['ln1_b'][:, j:j + 1],
                        ALU.mult, ALU.add)

            # P2: SA Q/K (c=0 halves) + V, so stream-0 softmax starts early
            wt = load_w_full(wd['sa_wq'], 0, 1024)
            proj_stream(wt, bias_t['sa_bq'], A, C, 0)
            wt = load_w_full(wd['sa_wk'], 0, 1024)
            proj_stream(wt, bias_t['sa_bk'], A, Hb, 0)
            wt = load_w_full(wd['sa_wv'], 0, 1024)
            vproj_full(wt, A, bcv_sb['sa_'], vg)

            # P3: stream-0 SA softmax; encoder stripes interleaved as PE fill
            attn_stream(0, C, Hb, vg, True, Q2, fill=enc_stripe)

            # P4: CA K/V projections from encT (fill SA softmax PE gaps);
            # must be emitted before SA-O(0) overwrites E's c0 columns
            wt = load_w_full(wd['ca_wk'], 0, 1024)
            proj_full(wt, bias_t['ca_bk'], E, Bb)
            wt = load_w_full(wd['ca_wv'], 0, 1024)
            vproj_full(wt, E, bcv_sb['ca_'], vg2)

            # P6: stream-0 post-SA chain up to CA softmax
            wt = load_w_full(wd['sa_wo'], 0, 1024)
            proj_stream(wt, bias_t['sa_bo'], Q2, E, 0, residual=A)
            ln_stream(E, bias_t['ln2_g'], bias_t['ln2_b'], 0)
            wt = load_w_full(wd['ca_wq'], 0, 1024)
            proj_stream(wt, bias_t['ca_bq'], E, Q2, 0)

            # P7: stream-0 CA softmax, interleaved with SA Q/K c=1 chunks
            # (PE fill while ACT runs the exps)
            wq1 = load_w_full(wd['sa_wq'], 0, 1024)
            wk1 = load_w_full(wd['sa_wk'], 0, 1024)

            def fill_qk1(hp):
                proj_m(wq1, bias_t['sa_bq'], A, C, 1, hp)
                proj_m(wk1, bias_t['sa_bk'], A, Hb, 1, hp)

            attn_stream(0, Q2, Bb, vg2, False, A, fill=fill_qk1)

            # P8: stream-1 SA softmax, interleaved with CA-O(0) chunks
            wo0 = load_w_full(wd['ca_wo'], 0, 1024)

            def fill_cao0(hp):
                proj_m(wo0, bias_t['ca_bo'], A, C, 0, hp, residual=E)

            attn_stream(1, C, Hb, vg, True, Q2, fill=fill_cao0)
            ln_stream(C, bias_t['ln3_g'], bias_t['ln3_b'], 0)

            # P10: stream-1 post-SA chain
            wt = load_w_full(wd['sa_wo'], 0, 1024)
            proj_stream(wt, bias_t['sa_bo'], Q2, E, 1, residual=A)
            ln_stream(E, bias_t['ln2_g'], bias_t['ln2_b'], 1)
            wt = load_w_full(wd['ca_wq'], 0, 1024)
            proj_stream(wt, bias_t['ca_bq'], E, Q2, 1)

            # P11: stream-1 CA softmax interleaved with FFN(0) chunks
            attn_stream(1, Q2, Bb, vg2, False, A,
                        fill=lambda hp: ffn_chunk(0, hp))

            # P12: stream-1 tail
            wt = load_w_full(wd['ca_wo'], 0, 1024)
            proj_stream(wt, bias_t['ca_bo'], A, C, 1, residual=E)
            ln_stream(C, bias_t['ln3_g'], bias_t['ln3_b'], 1)
            out_stream(0)
            ffn_stream(1)
            out_stream(1)

        if iters == 1:
            block_body()
        else:
            with tc.For_i(0, iters, 1):
                block_body()


_CACHE = {}


def _get_nc(iters=1):
    if iters not in _CACHE:
        _CACHE[iters] = _build(iters)
    return _CACHE[iters]


def _in_maps(inputs):
    shared = {}
    for n in W_NAMES + ['ffn_w1', 'ffn_w2']:
        shared[n] = np.ascontiguousarray(
            np.asarray(inputs[n], dtype=np.float32).astype(ml_dtypes.bfloat16))
    for n in B_NAMES + LN_NAMES + ['ffn_b1', 'ffn_b2']:
        shared[n] = np.ascontiguousarray(np.asarray(inputs[n], dtype=np.float32))
    for n in ('sa_bv', 'ca_bv'):
        shared[n] = shared[n].reshape(1, D)
    dec = np.asarray(inputs['decoder'], dtype=np.float32)
    enc = np.asarray(inputs['encoder'], dtype=np.float32)
    maps = []
    for b in range(B):
        m = dict(shared)
        m['decoder'] = np.ascontiguousarray(dec[b])
        m['encoder'] = np.ascontiguousarray(enc[b])
        maps.append(m)
    return maps


def kernel(**inputs):
    nc = _get_nc(1)
    res = run_bass_kernel_spmd(nc, _in_maps(inputs), core_ids=list(range(B)))
    return np.stack([res.results[b]['out'] for b in range(B)], axis=0)
